# revision 24
# baseline (speedup 1.0000x reference)
"""Causal attention (B=4, S=2048, D=1024, single head) on 8 TRN2 NeuronCores.

Sharding: data-parallel over batch x causal-balanced query split.
  core c -> batch b = c//2, role r = c%2.
  Queries: role r takes the 128-row tiles t with t%2 == r (t = 0..15).
  Local tile j (j = 0..7) is global tile t = 2j+r; its causal key need is
  128(t+1) <= 256(j+1), so both roles visit exactly j+1 key chunks of 256
  for local tile j: one SPMD program, near-zero padding (12.5% overcount
  vs the exact causal triangle, down from 18% at 512-key granularity).
  K/V: each core projects only its half of the sequence (role 0 rows
  0:1024, role 1 rows 1024:2048) and the halves are exchanged pair-wise
  through pair-shared DRAM with tiny AllGather barriers as rendezvous.

Per-core differences (which query rows, which keys are causally visible)
are carried in input data only: xqt/xth are host-sliced columns of x^T,
qidx holds each local query row's global index, and causality is a
data-driven additive mask (-1e6 where kpos > qidx). The mask is needed
only on the final (diagonal) 256-chunk of each tile; the DVE writes it
into PSUM before the score matmuls, which accumulate onto it with
start=False - no post-add on the critical path.

Weights are host-retiled to [ec, p, dc, e'] so every 128-column chunk of
W^T is one contiguous 256KB DMA (2KB per partition row).

Compute is bf16 on the TensorEngine with f32 PSUM accumulation; softmax
skips the running max (logits are ~N(0,1) after the 1/32 scale; masked
lanes sit at -31250 and underflow to exactly 0).
"""

import sys

if "/opt/trn_rl_repo" not in sys.path:
    sys.path.insert(0, "/opt/trn_rl_repo")

import ml_dtypes
import numpy as np

import bass_rust

import concourse.bass as bass
import concourse.mybir as mybir
from concourse.masks import make_identity
from concourse.tile import TileContext
from concourse.tile_rust import add_dep_helper

B, S, D = 4, 2048, 1024
P = 128
NCORES = 8
DC = D // P           # 8 contraction chunks of 128
QROWS = S // 2        # 1024 query rows per core
QT = QROWS // P       # 8 query tiles of 128 rows
SH = S // 2           # this core's K/V half
KBLK = 512            # key block size (exchange granularity)
KCH = 256             # score chunk size (causal granularity)
NKB = S // KBLK       # 4 key blocks
SCALE = 1.0 / np.sqrt(np.float32(D))
MASK_NEG = -1.0e6
GROUPS = [[0, 1], [2, 3], [4, 5], [6, 7]]
# Processing order: starts with a small tile (only needs key block 0, so
# attention can begin on the first exchange), ends with the smallest
# (nvis=1) so the post-PE tail (exp/transpose/ctx/norm/out-DMA of the
# final tile) is as short as possible.
PERM = (1, 3, 2, 5, 4, 7, 6, 0)

F32 = mybir.dt.float32
BF16 = mybir.dt.bfloat16


# ---------------------------------------------------------------------------
# This container's walrus build (setupSyncWait, CoreV2/V3GenImpl.cpp) rejects
# any instruction carrying more than one sem wait. Tile's wait-assignment
# freely emits several. Hoist all but one wait of each instruction onto NOPs
# inserted immediately before it on the same engine — the engine executes its
# stream in order, so waiting on a preceding same-engine NOP is equivalent.
def _split_multi_waits(nc):
    n_split = 0
    for fn in nc.m.functions:
        for bb in fn.blocks:
            insts = list(bb.instructions)
            out = []
            changed = False
            for inst in insts:
                si = inst.sync_info
                if si is not None and len(si.on_wait) > 1:
                    waits = list(si.on_wait)
                    for w in waits[:-1]:
                        nop = mybir.InstNoOp(
                            name=f"{inst.name}-wsplit{n_split}", ins=[], outs=[]
                        )
                        n_split += 1
                        nop.engine = inst.engine
                        nop.sync_info = bass_rust.SyncInfo(
                            on_wait=[w], on_update=[]
                        )
                        out.append(nop)
                    inst.sync_info = bass_rust.SyncInfo(
                        on_wait=[waits[-1]], on_update=list(si.on_update)
                    )
                    changed = True
                if si is not None and len(si.on_update) > 2:
                    raise RuntimeError(
                        f"{inst.name}: {len(si.on_update)} sync updates; "
                        "update-splitting not implemented"
                    )
                out.append(inst)
            if changed:
                bb.instructions = out
    return nc
# ---------------------------------------------------------------------------


def _build_nc():
    nc = bass.Bass()

    xth = nc.declare_dram_parameter("xth", [D, SH], BF16, isOutput=False)
    xqt = nc.declare_dram_parameter("xqt", [D, QROWS], BF16, isOutput=False)
    # weights host-retiled to [ec, p, dc, e']: chunk ec is contiguous
    wq = nc.declare_dram_parameter("wq", [DC, P, DC, P], BF16, isOutput=False)
    wk = nc.declare_dram_parameter("wk", [DC, P, DC, P], BF16, isOutput=False)
    wv = nc.declare_dram_parameter("wv", [DC, P, DC, P], BF16, isOutput=False)
    qidx = nc.declare_dram_parameter("qidx", [QROWS], F32, isOutput=False)
    rk = nc.declare_dram_parameter("rk", [1, 1], mybir.dt.uint32, isOutput=False)
    out = nc.declare_dram_parameter("out", [QROWS, D], BF16, isOutput=True)

    xth_r = xth.rearrange("(dc p) s -> p dc s", p=P)
    xqt_r = xqt.rearrange("(dc p) s -> p dc s", p=P)
    wq_r = wq.rearrange("ec p dc e -> p ec dc e")
    wk_r = wk.rearrange("ec p dc e -> p ec dc e")
    wv_r = wv.rearrange("ec p dc e -> p ec dc e")
    qidx_r = qidx.rearrange("(t p) -> p t", p=P)

    with TileContext(nc) as tc:
        # The race-detector sim can't model pair-aliased Shared DRAM (it
        # demands a single writer); ordering for the shared exchange is
        # enforced with explicit deps instead.
        tc.race_detector_enabled = False

        # Long-lived tiles. K^T / V are per-key-block so attention only
        # waits on the specific block's collective, not the whole tensor.
        persist = tc.alloc_tile_pool(name="persist", bufs=1)
        qt_sb = persist.tile([P, DC, QROWS], BF16, tag="qt_sb")   # Q^T [e, q]
        kt_b = [
            persist.tile([P, DC, KBLK], BF16, tag=f"kt_b{v}", name=f"kt_b{v}")
            for v in range(NKB)
        ]
        v_b = [
            persist.tile([P, KBLK // P, D], BF16, tag=f"v_b{v}", name=f"v_b{v}")
            for v in range(NKB)
        ]
        kpos_f = persist.tile([P, S], F32, tag="kpos_f")
        qidx_sb = persist.tile([P, QT], F32, tag="qidx_sb")
        ident = persist.tile([P, P], BF16, tag="ident")

        make_identity(nc, ident[:])

        # ---- Phase 1: projections + pair-wise K/V exchange ----
        with (
            tc.tile_pool(name="proj_in", bufs=1) as proj_in,
            tc.tile_pool(name="proj_w", bufs=2) as proj_w,
            tc.tile_pool(name="proj_st", bufs=2) as proj_st,
            tc.tile_pool(name="proj_ps", bufs=6, space="PSUM") as proj_ps,
            tc.tile_pool(name="cc_dram", bufs=1, space="DRAM") as cc_dram,
        ):
            # NOTE: no warmup collective. The runtime prepends its own init
            # collective (~21-50us on the trace clock) which already absorbs
            # the ncfw boot; with the ~23us min collective spacing, an extra
            # warmup would only push the real rendezvous barrier later.
            xth_sb = proj_in.tile([P, DC, SH], BF16, tag="xth_sb")
            xqt_sb = proj_in.tile([P, DC, QROWS], BF16, tag="xqt_sb")

            # iota values < 2048 are exact in f32
            nc.gpsimd.iota(
                kpos_f[:], pattern=[[1, S]], base=0, channel_multiplier=0,
                allow_small_or_imprecise_dtypes=True,
            )

            # DMA descriptors are packetized across all 16 DMA engines, so
            # per-descriptor drain is fast — the limiter is descriptor
            # POSTING (~640ns each on the issuing engine). Post from BOTH
            # HWDGE engines (Sync and Scalar) in parallel, few large
            # descriptors, in first-use order: the first matmul group
            # (KTh0, et=0) needs wk chunk 0 plus xth half 0.
            wk_sb = proj_w.tile([P, DC, DC, P], BF16, tag="w", bufs=3)
            wv_sb = proj_w.tile([P, DC, DC, P], BF16, tag="w", bufs=3)
            wq_sb = proj_w.tile([P, DC, DC, P], BF16, tag="w", bufs=3)
            # Post plan (both HWDGE engines, ordered by consumption
            # deadline). Sync carries the K-path inputs, the bulk tensors,
            # and the exchange; bulk posts are slotted BETWEEN the
            # staging-gated exchange writes so nothing critical queues
            # behind a multi-us queue-full stall. Scalar posts only the
            # light early chunks — it also runs half the staging copies.
            nc.sync.dma_start(wk_sb[:, 0], wk_r[:, 0])
            nc.sync.dma_start(xth_sb[:, 0:2, 0:KBLK], xth_r[:, 0:2, 0:KBLK])
            nc.sync.dma_start(xth_sb[:, 2:4, 0:KBLK], xth_r[:, 2:4, 0:KBLK])
            for et in (3, 5, 7):
                nc.sync.dma_start(wk_sb[:, et], wk_r[:, et])
            for dc in range(0, DC, 4):
                nc.sync.dma_start(
                    xth_sb[:, dc : dc + 4, KBLK:SH], xth_r[:, dc : dc + 4, KBLK:SH]
                )
            for et in range(0, DC, 2):
                nc.sync.dma_start(wv_sb[:, et : et + 2], wv_r[:, et : et + 2])
            # scalar stream (runs concurrently with the sync one)
            nc.scalar.dma_start(xth_sb[:, 4:6, 0:KBLK], xth_r[:, 4:6, 0:KBLK])
            nc.scalar.dma_start(xth_sb[:, 6:8, 0:KBLK], xth_r[:, 6:8, 0:KBLK])
            for et in (1, 2, 4, 6):
                nc.scalar.dma_start(wk_sb[:, et], wk_r[:, et])
            nc.scalar.dma_start(qidx_sb[:], qidx_r)

            # K^T/V halves are exchanged through pair-shared DRAM (cores
            # 2k/2k+1 alias addr_space="Shared" allocations): each core
            # DMA-writes its stagings into its rank's slot (runtime branch
            # on the rank register — the only non-data-driven role split),
            # one tiny AllGather acts as the pair rendezvous, then both
            # halves are DMA-read back at full bandwidth. This replaces 4
            # slow data collectives (~20us/MB) with plain DMA.
            # PSUM->SBUF staging copies alternate Scalar/Vector so neither
            # engine's backlog stalls the PE's PSUM-bank rotation.
            def _copy(i, dst, src):
                if i % 2 == 0:
                    nc.scalar.copy(dst, src)
                else:
                    nc.vector.tensor_copy(dst, src)

            def v_half(h):
                vst = proj_st.tile(
                    [P, KBLK // P, D], BF16, tag=f"vst{h}", name=f"vst{h}", bufs=1
                )
                for st in range(KBLK // P):
                    for ec in range(D // KBLK):
                        ps = proj_ps.tile([P, KBLK], F32, tag="proj_ps")
                        for dc in range(DC):
                            nc.tensor.matmul(
                                ps[:],
                                xth_sb[:, dc, h * KBLK + st * P : h * KBLK + (st + 1) * P],
                                wv_sb[:, 4 * ec : 4 * ec + 4, dc, :],
                                start=(dc == 0),
                                stop=(dc == DC - 1),
                            )
                        _copy(
                            2 * st + ec, vst[:, st, ec * KBLK : (ec + 1) * KBLK], ps[:]
                        )
                return vst

            def kt_half(h):
                ssl = slice(h * KBLK, (h + 1) * KBLK)
                ktst = proj_st.tile(
                    [P, DC, KBLK], BF16, tag=f"ktst{h}", name=f"ktst{h}", bufs=1
                )
                for et in range(DC):
                    ps = proj_ps.tile([P, KBLK], F32, tag="proj_ps")
                    for dc in range(DC):
                        nc.tensor.matmul(
                            ps[:],
                            wk_sb[:, et, dc, :],
                            xth_sb[:, dc, ssl],
                            start=(dc == 0),
                            stop=(dc == DC - 1),
                        )
                    _copy(et, ktst[:, et, :], ps[:])
                return ktst

            # One Shared tensor per (rank, slot) — the scheduler sim demands
            # a single writer inst per Shared DRAM tensor. Slots: 0=KTh0,
            # 1=Vh0, 2=KTh1, 3=Vh1 (flat 512K bf16 each).
            sh_d = [
                [
                    cc_dram.tile(
                        [D * KBLK], BF16, tag=f"sh_d{r}{j}",
                        name=f"sh_d{r}{j}", addr_space="Shared",
                    )
                    for j in range(4)
                ]
                for r in range(2)
            ]

            def kt_view(flat):
                return flat.rearrange("(et p s) -> p et s", p=P, s=KBLK)

            def v_view(flat):
                return flat.rearrange("(st p e) -> p st e", p=P, e=D)

            rk_reg = nc.sync.alloc_register("rk_reg")
            nc.sync.reg_load(rk_reg, rk[0:1, 0:1])
            # snap once per engine and reuse: each fresh snap==r expression
            # allocates registers, and the SP engine's pool is small.
            rk_is = {
                (id(nc.sync), r): nc.sync.snap(rk_reg) == r for r in range(2)
            }

            # Each staging is written to its rank's shared slot AS SOON as
            # it is produced (rank-predicated DMA pair — a skipped DMA
            # still increments its semaphore), so the single rendezvous
            # barrier only waits for the LAST staging's write, not 4MB of
            # serialized traffic at the end.
            ex_writes = {}

            def stage_out(st, view, slot):
                ex_writes[slot] = [
                    nc.sync.dma_start(
                        view(sh_d[r][slot]), st[:],
                        cond=rk_is[(id(nc.sync), r)],
                    )
                    for r in range(2)
                ]

            ktst0 = kt_half(0)
            stage_out(ktst0, kt_view, 0)
            # bulk posts AFTER the kt0 write in sync's stream: they post
            # ~31us (once kt0 is staged), well before their ~60us deadline
            for et in range(0, DC, 2):
                nc.sync.dma_start(wq_sb[:, et : et + 2], wq_r[:, et : et + 2])
            ktst1 = kt_half(1)
            stage_out(ktst1, kt_view, 2)
            nc.sync.dma_start(xqt_sb[:, 0:4, :], xqt_r[:, 0:4, :])
            nc.sync.dma_start(xqt_sb[:, 4:8, :], xqt_r[:, 4:8, :])
            vst0 = v_half(0)
            stage_out(vst0, v_view, 1)
            vst1 = v_half(1)
            stage_out(vst1, v_view, 3)

            # ONE rendezvous barrier: a single collective avoids the ~23us
            # min-spacing penalty a second one would pay (plus there is a
            # ~13us fixed issue latency per collective).
            b_in = cc_dram.tile([16], F32, tag="b_in_kv", name="b_in_kv")
            b_out = cc_dram.tile([2, 16], F32, tag="b_out_kv", name="b_out_kv")
            cc = nc.gpsimd.collective_compute(
                "AllGather",
                mybir.AluOpType.bypass,
                replica_groups=GROUPS,
                ins=[b_in[:]],
                outs=[b_out[:]],
            )
            for ws in ex_writes.values():
                for w in ws:
                    add_dep_helper(cc.ins, w.ins, True, "barrier after writes")
            # Reads ordered by attention's first-use (kt0,v0 first; v3
            # last), split across both HWDGE queues, so early tiles can
            # start while later blocks are still in flight.
            # sh_d[r][slot]: slot 0 = rank r's KT half, slot 2 = same for
            # the OTHER half pair... (slots: 0=KTh0, 1=Vh0, 2=KTh1, 3=Vh1;
            # rank r wrote its own halves to sh_d[r][*]). Global block for
            # (rank, h): kt_b[2*rank + h].
            reads = (
                (kt_view, 0, 0, 0),   # rank0 KTh0 -> kt_b[0]
                (v_view, 0, 1, 0),    # rank0 Vh0  -> v_b[0]
                (kt_view, 0, 2, 1),   # rank0 KTh1 -> kt_b[1]
                (v_view, 0, 3, 1),    # rank0 Vh1  -> v_b[1]
                (kt_view, 1, 0, 2),   # rank1 KTh0 -> kt_b[2]
                (v_view, 1, 1, 2),    # rank1 Vh0  -> v_b[2]
                (kt_view, 1, 2, 3),   # rank1 KTh1 -> kt_b[3]
                (v_view, 1, 3, 3),    # rank1 Vh1  -> v_b[3]
            )
            for ri, (view, rank, slot, g) in enumerate(reads):
                dst = kt_b if view is kt_view else v_b
                eng = nc.sync if ri % 2 == 0 else nc.scalar
                rd = eng.dma_start(dst[g][:], view(sh_d[rank][slot]))
                add_dep_helper(rd.ins, cc.ins, True, "read after rdv")

            # Q^T [e, q] = Wq^T @ xq^T (overlaps the second collective).
            for et in range(DC):
                for sc in range(QROWS // KBLK):
                    ps = proj_ps.tile([P, KBLK], F32, tag="proj_ps")
                    for dc in range(DC):
                        nc.tensor.matmul(
                            ps[:],
                            wq_sb[:, et, dc, :],
                            xqt_sb[:, dc, sc * KBLK : (sc + 1) * KBLK],
                            start=(dc == 0),
                            stop=(dc == DC - 1),
                        )
                    nc.scalar.copy(qt_sb[:, et, sc * KBLK : (sc + 1) * KBLK], ps[:])

        # ---- Phase 2: block attention ----
        # Local tile j visits j+1 chunks of 256 keys; only the last
        # (diagonal) chunk needs the causal mask, pre-written into PSUM.
        with (
            tc.tile_pool(name="att", bufs=2) as att,
            tc.tile_pool(name="att_sm", bufs=3) as att_sm,
            tc.tile_pool(name="ps_sc", bufs=2, space="PSUM") as ps_sc,
            tc.tile_pool(name="ps_pt", bufs=2, space="PSUM") as ps_pt,
            tc.tile_pool(name="ps_ctx", bufs=2, space="PSUM") as ps_ctx,
        ):
            for j in PERM:
                nch = j + 1           # 256-key score chunks
                nkc = 2 * nch         # 128-key ctx chunks
                p_sb = att.tile([P, S], BF16, tag="p_sb")
                pt_sb = att.tile([P, S // P, P], BF16, tag="pt_sb")
                sums = att_sm.tile([P, QT], F32, tag="sums")
                qcol = qidx_sb[:, j : j + 1]

                for c in range(nch):
                    ksl = slice(c * KCH, (c + 1) * KCH)
                    sc_ps = ps_sc.tile([P, KCH], F32, tag="sc_ps")
                    diag = c == j
                    if diag:
                        # bias = (kpos > qidx) * -1e6, written into PSUM;
                        # score matmuls accumulate onto it (start=False).
                        nc.vector.tensor_scalar(
                            sc_ps[:], kpos_f[:, ksl], qcol, MASK_NEG,
                            mybir.AluOpType.is_gt, mybir.AluOpType.mult,
                        )
                    for ec in range(DC):
                        nc.tensor.matmul(
                            sc_ps[:],
                            qt_sb[:, ec, j * P : (j + 1) * P],
                            kt_b[c // 2][:, ec, (c % 2) * KCH : (c % 2 + 1) * KCH],
                            start=(ec == 0 and not diag),
                            stop=(ec == DC - 1),
                            skip_group_check=diag,
                        )
                    nc.scalar.activation(
                        p_sb[:, ksl], sc_ps[:],
                        mybir.ActivationFunctionType.Exp,
                        scale=float(SCALE),
                        accum_out=sums[:, c : c + 1],
                    )

                # Transposes batched 4 per PSUM tile with one DVE copy per
                # batch: keeps the PE at its ~110ns/transpose issue rate
                # instead of serializing on per-transpose copy+semaphores.
                for g in range(0, nkc, 4):
                    gsz = min(4, nkc - g)
                    pt_ps = ps_pt.tile([P, 4, P], BF16, tag="pt_ps")
                    for i in range(gsz):
                        kc = g + i
                        nc.tensor.transpose(
                            pt_ps[:, i, :], p_sb[:, kc * P : (kc + 1) * P], ident[:]
                        )
                    nc.vector.tensor_copy(
                        pt_sb[:, g : g + gsz, :], pt_ps[:, 0:gsz, :]
                    )

                tot = att_sm.tile([P, 1], F32, tag="tot")
                rinv = att_sm.tile([P, 1], F32, tag="rinv")
                nc.vector.reduce_sum(
                    tot[:], sums[:, :nch], axis=mybir.AxisListType.X
                )
                nc.vector.reciprocal(rinv[:], tot[:])

                ctx_lo = ps_ctx.tile([P, KBLK], F32, tag="ctx_lo")
                ctx_hi = ps_ctx.tile([P, KBLK], F32, tag="ctx_hi")
                for kc in range(nkc):
                    vb = v_b[kc // (KBLK // P)]
                    vrow = kc % (KBLK // P)
                    nc.tensor.matmul(
                        ctx_lo[:], pt_sb[:, kc, :], vb[:, vrow, 0:KBLK],
                        start=(kc == 0), stop=(kc == nkc - 1),
                    )
                    nc.tensor.matmul(
                        ctx_hi[:], pt_sb[:, kc, :], vb[:, vrow, KBLK:D],
                        start=(kc == 0), stop=(kc == nkc - 1),
                    )

                # Normalize lo on DVE, hi on Scalar (Copy with per-partition
                # scale), each half DMA'd out as soon as it's ready.
                out_sb = att.tile([P, D], BF16, tag="out_sb")
                nc.vector.tensor_scalar_mul(out_sb[:, 0:KBLK], ctx_lo[:], rinv[:])
                nc.sync.dma_start(out[j * P : (j + 1) * P, 0:KBLK], out_sb[:, 0:KBLK])
                nc.scalar.activation(
                    out_sb[:, KBLK:D], ctx_hi[:],
                    mybir.ActivationFunctionType.Copy, scale=rinv[:],
                )
                nc.scalar.dma_start(
                    out[j * P : (j + 1) * P, KBLK:D], out_sb[:, KBLK:D]
                )

        persist.release()

    return _split_multi_waits(nc)


_NC_CACHE = None


def _get_nc():
    global _NC_CACHE
    if _NC_CACHE is None:
        _NC_CACHE = _build_nc()
    return _NC_CACHE


def _qrows(role):
    # local tile j = global 128-row tile 2j+role, j ascending
    return np.concatenate(
        [np.arange(t * P, (t + 1) * P) for t in range(role, 2 * QT, 2)]
    )


def _retile_w(W):
    # [d, e] -> [ec, p, dc, e']: chunk ec contiguous, 2KB per partition row
    return np.ascontiguousarray(
        W.astype(ml_dtypes.bfloat16).reshape(DC, P, DC, P).transpose(2, 1, 0, 3)
    )


def _shard_inputs(x, Wq, Wk, Wv):
    bf = ml_dtypes.bfloat16
    w = {
        "wq": _retile_w(Wq),
        "wk": _retile_w(Wk),
        "wv": _retile_w(Wv),
    }
    in_maps = []
    for c in range(NCORES):
        b, r = c // 2, c % 2
        rows = _qrows(r)
        xbT = x[b].T.astype(bf)                                  # [D, S]
        in_maps.append(
            {
                "xth": np.ascontiguousarray(xbT[:, r * SH : (r + 1) * SH]),
                "xqt": np.ascontiguousarray(xbT[:, rows]),
                "qidx": rows.astype(np.float32),
                "rk": np.array([[r]], dtype=np.uint32),
                **w,
            }
        )
    return in_maps


def _unshard(results, dtype):
    out = np.empty((B, S, D), dtype=dtype)
    for c in range(NCORES):
        b, r = c // 2, c % 2
        out[b, _qrows(r), :] = results[c]["out"].astype(dtype)
    return out


def run(x, Wq, Wk, Wv, trace=False, tmpdir=None):
    from concourse.bass_utils import run_bass_kernel_spmd

    nc = _get_nc()
    in_maps = _shard_inputs(x, Wq, Wk, Wv)
    res = run_bass_kernel_spmd(
        nc, in_maps, core_ids=list(range(NCORES)), trace=trace, tmpdir=tmpdir
    )
    return _unshard(res.results, np.dtype(x.dtype)), res


def kernel(x, Wq, Wk, Wv):
    out, _ = run(np.asarray(x), np.asarray(Wq), np.asarray(Wk), np.asarray(Wv))
    return out


# revision 26
# speedup vs baseline: 1.1085x; 1.1085x over previous
"""Causal attention (B=4, S=2048, D=1024, single head) on 8 TRN2 NeuronCores.

Sharding: data-parallel over batch x causal-balanced query split.
  core c -> batch b = c//2, role r = c%2.
  Queries: role r takes the 128-row tiles t with t%2 == r (t = 0..15).
  Local tile j (j = 0..7) is global tile t = 2j+r; its causal key need is
  128(t+1) <= 256(j+1), so both roles visit exactly j+1 key chunks of 256
  for local tile j: one SPMD program, near-zero padding (12.5% overcount
  vs the exact causal triangle, down from 18% at 512-key granularity).
  K/V: each core projects only its half of the sequence (role 0 rows
  0:1024, role 1 rows 1024:2048) and the halves are exchanged pair-wise
  through pair-shared DRAM with tiny AllGather barriers as rendezvous.

Per-core differences (which query rows, which keys are causally visible)
are carried in input data only: xqt/xth are host-sliced columns of x^T,
qidx holds each local query row's global index, and causality is a
data-driven additive mask (-1e6 where kpos > qidx). The mask is needed
only on the final (diagonal) 256-chunk of each tile; the DVE writes it
into PSUM before the score matmuls, which accumulate onto it with
start=False - no post-add on the critical path.

Weights are host-retiled to [ec, p, dc, e'] so every 128-column chunk of
W^T is one contiguous 256KB DMA (2KB per partition row).

Compute is bf16 on the TensorEngine with f32 PSUM accumulation; softmax
skips the running max (logits are ~N(0,1) after the 1/32 scale; masked
lanes sit at -31250 and underflow to exactly 0).
"""

import sys

if "/opt/trn_rl_repo" not in sys.path:
    sys.path.insert(0, "/opt/trn_rl_repo")

import ml_dtypes
import numpy as np

import bass_rust

import concourse.bass as bass
import concourse.mybir as mybir
from concourse.masks import make_identity
from concourse.tile import TileContext
from concourse.tile_rust import add_dep_helper

B, S, D = 4, 2048, 1024
P = 128
NCORES = 8
DC = D // P           # 8 contraction chunks of 128
QROWS = S // 2        # 1024 query rows per core
QT = QROWS // P       # 8 query tiles of 128 rows
SH = S // 2           # this core's K/V half
KBLK = 512            # key block size (exchange granularity)
KCH = 256             # score chunk size (causal granularity)
NKB = S // KBLK       # 4 key blocks
SCALE = 1.0 / np.sqrt(np.float32(D))
MASK_NEG = -1.0e6
GROUPS = [[0, 1], [2, 3], [4, 5], [6, 7]]
# Processing order: starts with a small tile (only needs key block 0, so
# attention can begin on the first exchange), ends with the smallest
# (nvis=1) so the post-PE tail (exp/transpose/ctx/norm/out-DMA of the
# final tile) is as short as possible.
PERM = (1, 0, 3, 5, 7, 6, 4, 2)

F32 = mybir.dt.float32
BF16 = mybir.dt.bfloat16


# ---------------------------------------------------------------------------
# This container's walrus build (setupSyncWait, CoreV2/V3GenImpl.cpp) rejects
# any instruction carrying more than one sem wait. Tile's wait-assignment
# freely emits several. Hoist all but one wait of each instruction onto NOPs
# inserted immediately before it on the same engine — the engine executes its
# stream in order, so waiting on a preceding same-engine NOP is equivalent.
def _split_multi_waits(nc):
    n_split = 0
    for fn in nc.m.functions:
        for bb in fn.blocks:
            insts = list(bb.instructions)
            out = []
            changed = False
            for inst in insts:
                si = inst.sync_info
                if si is not None and len(si.on_wait) > 1:
                    waits = list(si.on_wait)
                    for w in waits[:-1]:
                        nop = mybir.InstNoOp(
                            name=f"{inst.name}-wsplit{n_split}", ins=[], outs=[]
                        )
                        n_split += 1
                        nop.engine = inst.engine
                        nop.sync_info = bass_rust.SyncInfo(
                            on_wait=[w], on_update=[]
                        )
                        out.append(nop)
                    inst.sync_info = bass_rust.SyncInfo(
                        on_wait=[waits[-1]], on_update=list(si.on_update)
                    )
                    changed = True
                if si is not None and len(si.on_update) > 2:
                    raise RuntimeError(
                        f"{inst.name}: {len(si.on_update)} sync updates; "
                        "update-splitting not implemented"
                    )
                out.append(inst)
            if changed:
                bb.instructions = out
    return nc
# ---------------------------------------------------------------------------


def _build_nc():
    nc = bass.Bass()

    xth = nc.declare_dram_parameter("xth", [D, SH], BF16, isOutput=False)
    xqt = nc.declare_dram_parameter("xqt", [D, QROWS], BF16, isOutput=False)
    # weights host-retiled to [ec, p, dc, e']: chunk ec is contiguous
    wq = nc.declare_dram_parameter("wq", [DC, P, DC, P], BF16, isOutput=False)
    wk = nc.declare_dram_parameter("wk", [DC, P, DC, P], BF16, isOutput=False)
    wv = nc.declare_dram_parameter("wv", [DC, P, DC, P], BF16, isOutput=False)
    qidx = nc.declare_dram_parameter("qidx", [QROWS], F32, isOutput=False)
    rk = nc.declare_dram_parameter("rk", [1, 1], mybir.dt.uint32, isOutput=False)
    out = nc.declare_dram_parameter("out", [QROWS, D], BF16, isOutput=True)

    xth_r = xth.rearrange("(dc p) s -> p dc s", p=P)
    xqt_r = xqt.rearrange("(dc p) s -> p dc s", p=P)
    wq_r = wq.rearrange("ec p dc e -> p ec dc e")
    wk_r = wk.rearrange("ec p dc e -> p ec dc e")
    wv_r = wv.rearrange("ec p dc e -> p ec dc e")
    qidx_r = qidx.rearrange("(t p) -> p t", p=P)

    with TileContext(nc) as tc:
        # The race-detector sim can't model pair-aliased Shared DRAM (it
        # demands a single writer); ordering for the shared exchange is
        # enforced with explicit deps instead.
        tc.race_detector_enabled = False

        # Long-lived tiles. K^T / V are per-key-block so attention only
        # waits on the specific block's collective, not the whole tensor.
        persist = tc.alloc_tile_pool(name="persist", bufs=1)
        qt_sb = persist.tile([P, DC, QROWS], BF16, tag="qt_sb")   # Q^T [e, q]
        kt_b = [
            persist.tile([P, DC, KBLK], BF16, tag=f"kt_b{v}", name=f"kt_b{v}")
            for v in range(NKB)
        ]
        v_b = [
            persist.tile([P, KBLK // P, D], BF16, tag=f"v_b{v}", name=f"v_b{v}")
            for v in range(NKB)
        ]
        kpos_f = persist.tile([P, S], F32, tag="kpos_f")
        qidx_sb = persist.tile([P, QT], F32, tag="qidx_sb")
        ident = persist.tile([P, P], BF16, tag="ident")

        make_identity(nc, ident[:])

        # ---- Phase 1: projections + pair-wise K/V exchange ----
        with (
            tc.tile_pool(name="proj_in", bufs=1) as proj_in,
            tc.tile_pool(name="proj_w", bufs=2) as proj_w,
            tc.tile_pool(name="proj_st", bufs=2) as proj_st,
            tc.tile_pool(name="proj_ps", bufs=6, space="PSUM") as proj_ps,
            tc.tile_pool(name="cc_dram", bufs=1, space="DRAM") as cc_dram,
        ):
            # NOTE: no warmup collective. The runtime prepends its own init
            # collective (~21-50us on the trace clock) which already absorbs
            # the ncfw boot; with the ~23us min collective spacing, an extra
            # warmup would only push the real rendezvous barrier later.
            xth_sb = proj_in.tile([P, DC, SH], BF16, tag="xth_sb")
            xqt_sb = proj_in.tile([P, DC, QROWS], BF16, tag="xqt_sb")

            # iota values < 2048 are exact in f32
            nc.gpsimd.iota(
                kpos_f[:], pattern=[[1, S]], base=0, channel_multiplier=0,
                allow_small_or_imprecise_dtypes=True,
            )

            # DMA descriptors are packetized across all 16 DMA engines, so
            # per-descriptor drain is fast — the limiter is descriptor
            # POSTING (~640ns each on the issuing engine). Post from BOTH
            # HWDGE engines (Sync and Scalar) in parallel, few large
            # descriptors, in first-use order: the first matmul group
            # (KTh0, et=0) needs wk chunk 0 plus xth half 0.
            wk_sb = proj_w.tile([P, DC, DC, P], BF16, tag="w", bufs=3)
            wv_sb = proj_w.tile([P, DC, DC, P], BF16, tag="w", bufs=3)
            wq_sb = proj_w.tile([P, DC, DC, P], BF16, tag="w", bufs=3)
            # Post plan (both HWDGE engines, ordered by consumption
            # deadline). Sync carries the K-path inputs, the bulk tensors,
            # and the exchange; bulk posts are slotted BETWEEN the
            # staging-gated exchange writes so nothing critical queues
            # behind a multi-us queue-full stall. Scalar posts only the
            # light early chunks — it also runs half the staging copies.
            nc.sync.dma_start(wk_sb[:, 0], wk_r[:, 0])
            nc.sync.dma_start(xth_sb[:, 0:2, 0:KBLK], xth_r[:, 0:2, 0:KBLK])
            nc.sync.dma_start(xth_sb[:, 2:4, 0:KBLK], xth_r[:, 2:4, 0:KBLK])
            for et in (3, 5, 7):
                nc.sync.dma_start(wk_sb[:, et], wk_r[:, et])
            for dc in range(0, DC, 4):
                nc.sync.dma_start(
                    xth_sb[:, dc : dc + 4, KBLK:SH], xth_r[:, dc : dc + 4, KBLK:SH]
                )
            for et in range(0, DC, 2):
                nc.sync.dma_start(wv_sb[:, et : et + 2], wv_r[:, et : et + 2])
            # scalar stream (runs concurrently with the sync one)
            nc.scalar.dma_start(xth_sb[:, 4:6, 0:KBLK], xth_r[:, 4:6, 0:KBLK])
            nc.scalar.dma_start(xth_sb[:, 6:8, 0:KBLK], xth_r[:, 6:8, 0:KBLK])
            for et in (1, 2, 4, 6):
                nc.scalar.dma_start(wk_sb[:, et], wk_r[:, et])
            nc.scalar.dma_start(qidx_sb[:], qidx_r)

            # K^T/V halves are exchanged through pair-shared DRAM (cores
            # 2k/2k+1 alias addr_space="Shared" allocations): each core
            # DMA-writes its stagings into its rank's slot (runtime branch
            # on the rank register — the only non-data-driven role split),
            # one tiny AllGather acts as the pair rendezvous, then both
            # halves are DMA-read back at full bandwidth. This replaces 4
            # slow data collectives (~20us/MB) with plain DMA.
            # PSUM->SBUF staging copies alternate Scalar/Vector so neither
            # engine's backlog stalls the PE's PSUM-bank rotation.
            def _copy(i, dst, src):
                if i % 2 == 0:
                    nc.scalar.copy(dst, src)
                else:
                    nc.vector.tensor_copy(dst, src)

            def v_half(h):
                vst = proj_st.tile(
                    [P, KBLK // P, D], BF16, tag=f"vst{h}", name=f"vst{h}", bufs=1
                )
                for st in range(KBLK // P):
                    for ec in range(D // KBLK):
                        ps = proj_ps.tile([P, KBLK], F32, tag="proj_ps")
                        for dc in range(DC):
                            nc.tensor.matmul(
                                ps[:],
                                xth_sb[:, dc, h * KBLK + st * P : h * KBLK + (st + 1) * P],
                                wv_sb[:, 4 * ec : 4 * ec + 4, dc, :],
                                start=(dc == 0),
                                stop=(dc == DC - 1),
                            )
                        _copy(
                            2 * st + ec, vst[:, st, ec * KBLK : (ec + 1) * KBLK], ps[:]
                        )
                return vst

            def kt_half(h):
                ssl = slice(h * KBLK, (h + 1) * KBLK)
                ktst = proj_st.tile(
                    [P, DC, KBLK], BF16, tag=f"ktst{h}", name=f"ktst{h}", bufs=1
                )
                for et in range(DC):
                    ps = proj_ps.tile([P, KBLK], F32, tag="proj_ps")
                    for dc in range(DC):
                        nc.tensor.matmul(
                            ps[:],
                            wk_sb[:, et, dc, :],
                            xth_sb[:, dc, ssl],
                            start=(dc == 0),
                            stop=(dc == DC - 1),
                        )
                    _copy(et, ktst[:, et, :], ps[:])
                return ktst

            # One Shared tensor per (rank, slot) — the scheduler sim demands
            # a single writer inst per Shared DRAM tensor. Slots: 0=KTh0,
            # 1=Vh0, 2=KTh1, 3=Vh1 (flat 512K bf16 each).
            sh_d = [
                [
                    cc_dram.tile(
                        [D * KBLK], BF16, tag=f"sh_d{r}{j}",
                        name=f"sh_d{r}{j}", addr_space="Shared",
                    )
                    for j in range(4)
                ]
                for r in range(2)
            ]

            def kt_view(flat):
                return flat.rearrange("(et p s) -> p et s", p=P, s=KBLK)

            def v_view(flat):
                return flat.rearrange("(st p e) -> p st e", p=P, e=D)

            rk_reg = nc.sync.alloc_register("rk_reg")
            nc.sync.reg_load(rk_reg, rk[0:1, 0:1])
            # snap once per engine and reuse: each fresh snap==r expression
            # allocates registers, and the SP engine's pool is small.
            rk_is = {
                (id(nc.sync), r): nc.sync.snap(rk_reg) == r for r in range(2)
            }

            # Each staging is written to its rank's shared slot AS SOON as
            # it is produced (rank-predicated DMA pair — a skipped DMA
            # still increments its semaphore), so the single rendezvous
            # barrier only waits for the LAST staging's write, not 4MB of
            # serialized traffic at the end.
            ex_writes = {}

            def stage_out(st, view, slot):
                ex_writes[slot] = [
                    nc.sync.dma_start(
                        view(sh_d[r][slot]), st[:],
                        cond=rk_is[(id(nc.sync), r)],
                    )
                    for r in range(2)
                ]

            def barrier(name, slots):
                b_in = cc_dram.tile([16], F32, tag=f"b_in_{name}", name=f"b_in_{name}")
                b_out = cc_dram.tile(
                    [2, 16], F32, tag=f"b_out_{name}", name=f"b_out_{name}"
                )
                cc = nc.gpsimd.collective_compute(
                    "AllGather",
                    mybir.AluOpType.bypass,
                    replica_groups=GROUPS,
                    ins=[b_in[:]],
                    outs=[b_out[:]],
                )
                for slot in slots:
                    for w in ex_writes[slot]:
                        add_dep_helper(cc.ins, w.ins, True, "barrier after writes")
                return cc

            def read_back(cc, items):
                # items: (view, rank, slot, global block). Ordered by
                # attention's first use, split across both HWDGE queues.
                for ri, (view, rank, slot, g) in enumerate(items):
                    dst = kt_b[g] if view is kt_view else v_b[g]
                    eng = nc.sync if ri % 2 == 0 else nc.scalar
                    rd = eng.dma_start(dst[:], view(sh_d[rank][slot]))
                    add_dep_helper(rd.ins, cc.ins, True, "read after rdv")

            # Staggered exchange: {KTh0, Vh0} (global blocks 0 and 2) ride
            # an EARLY barrier fully hidden under the remaining stagings +
            # Q-projection; only {KTh1, Vh1} (blocks 1 and 3, 2MB) ride the
            # late one, so collective-duration jitter exposes at most a
            # couple of attention tiles.
            ktst0 = kt_half(0)
            stage_out(ktst0, kt_view, 0)
            for et in range(0, DC, 2):
                nc.sync.dma_start(wq_sb[:, et : et + 2], wq_r[:, et : et + 2])
            vst0 = v_half(0)
            stage_out(vst0, v_view, 1)
            cc1 = barrier("b1", (0, 1))
            read_back(cc1, (
                (kt_view, 0, 0, 0),
                (v_view, 0, 1, 0),
                (kt_view, 1, 0, 2),
                (v_view, 1, 1, 2),
            ))
            ktst1 = kt_half(1)
            stage_out(ktst1, kt_view, 2)
            nc.sync.dma_start(xqt_sb[:, 0:4, :], xqt_r[:, 0:4, :])
            nc.sync.dma_start(xqt_sb[:, 4:8, :], xqt_r[:, 4:8, :])
            vst1 = v_half(1)
            stage_out(vst1, v_view, 3)
            cc2 = barrier("b2", (2, 3))
            read_back(cc2, (
                (kt_view, 0, 2, 1),
                (v_view, 0, 3, 1),
                (kt_view, 1, 2, 3),
                (v_view, 1, 3, 3),
            ))

            # Q^T [e, q] = Wq^T @ xq^T (overlaps the second collective).
            for et in range(DC):
                for sc in range(QROWS // KBLK):
                    ps = proj_ps.tile([P, KBLK], F32, tag="proj_ps")
                    for dc in range(DC):
                        nc.tensor.matmul(
                            ps[:],
                            wq_sb[:, et, dc, :],
                            xqt_sb[:, dc, sc * KBLK : (sc + 1) * KBLK],
                            start=(dc == 0),
                            stop=(dc == DC - 1),
                        )
                    nc.scalar.copy(qt_sb[:, et, sc * KBLK : (sc + 1) * KBLK], ps[:])

        # ---- Phase 2: block attention ----
        # Local tile j visits j+1 chunks of 256 keys; only the last
        # (diagonal) chunk needs the causal mask, pre-written into PSUM.
        with (
            tc.tile_pool(name="att", bufs=2) as att,
            tc.tile_pool(name="att_sm", bufs=3) as att_sm,
            tc.tile_pool(name="ps_sc", bufs=2, space="PSUM") as ps_sc,
            tc.tile_pool(name="ps_pt", bufs=2, space="PSUM") as ps_pt,
            tc.tile_pool(name="ps_ctx", bufs=2, space="PSUM") as ps_ctx,
        ):
            for j in PERM:
                nch = j + 1           # 256-key score chunks
                nkc = 2 * nch         # 128-key ctx chunks
                p_sb = att.tile([P, S], BF16, tag="p_sb")
                pt_sb = att.tile([P, S // P, P], BF16, tag="pt_sb")
                sums = att_sm.tile([P, QT], F32, tag="sums")
                qcol = qidx_sb[:, j : j + 1]

                for c in range(nch):
                    ksl = slice(c * KCH, (c + 1) * KCH)
                    sc_ps = ps_sc.tile([P, KCH], F32, tag="sc_ps")
                    diag = c == j
                    if diag:
                        # bias = (kpos > qidx) * -1e6, written into PSUM;
                        # score matmuls accumulate onto it (start=False).
                        nc.vector.tensor_scalar(
                            sc_ps[:], kpos_f[:, ksl], qcol, MASK_NEG,
                            mybir.AluOpType.is_gt, mybir.AluOpType.mult,
                        )
                    for ec in range(DC):
                        nc.tensor.matmul(
                            sc_ps[:],
                            qt_sb[:, ec, j * P : (j + 1) * P],
                            kt_b[c // 2][:, ec, (c % 2) * KCH : (c % 2 + 1) * KCH],
                            start=(ec == 0 and not diag),
                            stop=(ec == DC - 1),
                            skip_group_check=diag,
                        )
                    nc.scalar.activation(
                        p_sb[:, ksl], sc_ps[:],
                        mybir.ActivationFunctionType.Exp,
                        scale=float(SCALE),
                        accum_out=sums[:, c : c + 1],
                    )

                # Transposes batched 4 per PSUM tile with one DVE copy per
                # batch: keeps the PE at its ~110ns/transpose issue rate
                # instead of serializing on per-transpose copy+semaphores.
                for g in range(0, nkc, 4):
                    gsz = min(4, nkc - g)
                    pt_ps = ps_pt.tile([P, 4, P], BF16, tag="pt_ps")
                    for i in range(gsz):
                        kc = g + i
                        nc.tensor.transpose(
                            pt_ps[:, i, :], p_sb[:, kc * P : (kc + 1) * P], ident[:]
                        )
                    nc.vector.tensor_copy(
                        pt_sb[:, g : g + gsz, :], pt_ps[:, 0:gsz, :]
                    )

                tot = att_sm.tile([P, 1], F32, tag="tot")
                rinv = att_sm.tile([P, 1], F32, tag="rinv")
                nc.vector.reduce_sum(
                    tot[:], sums[:, :nch], axis=mybir.AxisListType.X
                )
                nc.vector.reciprocal(rinv[:], tot[:])

                ctx_lo = ps_ctx.tile([P, KBLK], F32, tag="ctx_lo")
                ctx_hi = ps_ctx.tile([P, KBLK], F32, tag="ctx_hi")
                for kc in range(nkc):
                    vb = v_b[kc // (KBLK // P)]
                    vrow = kc % (KBLK // P)
                    nc.tensor.matmul(
                        ctx_lo[:], pt_sb[:, kc, :], vb[:, vrow, 0:KBLK],
                        start=(kc == 0), stop=(kc == nkc - 1),
                    )
                    nc.tensor.matmul(
                        ctx_hi[:], pt_sb[:, kc, :], vb[:, vrow, KBLK:D],
                        start=(kc == 0), stop=(kc == nkc - 1),
                    )

                # Normalize lo on DVE, hi on Scalar (Copy with per-partition
                # scale), each half DMA'd out as soon as it's ready.
                out_sb = att.tile([P, D], BF16, tag="out_sb")
                nc.vector.tensor_scalar_mul(out_sb[:, 0:KBLK], ctx_lo[:], rinv[:])
                nc.sync.dma_start(out[j * P : (j + 1) * P, 0:KBLK], out_sb[:, 0:KBLK])
                nc.scalar.activation(
                    out_sb[:, KBLK:D], ctx_hi[:],
                    mybir.ActivationFunctionType.Copy, scale=rinv[:],
                )
                nc.scalar.dma_start(
                    out[j * P : (j + 1) * P, KBLK:D], out_sb[:, KBLK:D]
                )

        persist.release()

    return _split_multi_waits(nc)


_NC_CACHE = None


def _get_nc():
    global _NC_CACHE
    if _NC_CACHE is None:
        _NC_CACHE = _build_nc()
    return _NC_CACHE


def _qrows(role):
    # local tile j = global 128-row tile 2j+role, j ascending
    return np.concatenate(
        [np.arange(t * P, (t + 1) * P) for t in range(role, 2 * QT, 2)]
    )


def _retile_w(W):
    # [d, e] -> [ec, p, dc, e']: chunk ec contiguous, 2KB per partition row
    return np.ascontiguousarray(
        W.astype(ml_dtypes.bfloat16).reshape(DC, P, DC, P).transpose(2, 1, 0, 3)
    )


def _shard_inputs(x, Wq, Wk, Wv):
    bf = ml_dtypes.bfloat16
    w = {
        "wq": _retile_w(Wq),
        "wk": _retile_w(Wk),
        "wv": _retile_w(Wv),
    }
    in_maps = []
    for c in range(NCORES):
        b, r = c // 2, c % 2
        rows = _qrows(r)
        xbT = x[b].T.astype(bf)                                  # [D, S]
        in_maps.append(
            {
                "xth": np.ascontiguousarray(xbT[:, r * SH : (r + 1) * SH]),
                "xqt": np.ascontiguousarray(xbT[:, rows]),
                "qidx": rows.astype(np.float32),
                "rk": np.array([[r]], dtype=np.uint32),
                **w,
            }
        )
    return in_maps


def _unshard(results, dtype):
    out = np.empty((B, S, D), dtype=dtype)
    for c in range(NCORES):
        b, r = c // 2, c % 2
        out[b, _qrows(r), :] = results[c]["out"].astype(dtype)
    return out


def run(x, Wq, Wk, Wv, trace=False, tmpdir=None):
    from concourse.bass_utils import run_bass_kernel_spmd

    nc = _get_nc()
    in_maps = _shard_inputs(x, Wq, Wk, Wv)
    res = run_bass_kernel_spmd(
        nc, in_maps, core_ids=list(range(NCORES)), trace=trace, tmpdir=tmpdir
    )
    return _unshard(res.results, np.dtype(x.dtype)), res


def kernel(x, Wq, Wk, Wv):
    out, _ = run(np.asarray(x), np.asarray(Wq), np.asarray(Wk), np.asarray(Wv))
    return out
